# revision 6
# baseline (speedup 1.0000x reference)
"""PolynomialToRoots on 8 Trainium2 NeuronCores.

Finds all 32 roots of 32768 degree-32 real polynomials (companion-matrix
eigenvalues) with a batched, guarded Ehrlich-Aberth iteration run entirely
on-device, data-parallel over the batch axis (4096 polynomials per core).

Output roots are conjugate-symmetrized and canonically sorted per row
(eigenvalue order of LAPACK geev is implementation-defined, so any
order-invariant comparison must sort; we return a deterministic canonical
order).
"""
import numpy as np

M = 32                     # polynomial degree / roots per row
P = 128                    # SBUF partitions
G = 16                     # row-groups per chunk
F = G * M                  # free dim: 512
CH = 2                     # chunks per core
ROWS_CORE = CH * G * P     # 4096
NCORES = 8
B = ROWS_CORE * NCORES     # 32768
NITER = 22
EPS = 1e-36
SEPS = 1e-30

_CACHE = {}


def _get_nc():
    if "nc" in _CACHE:
        return _CACHE["nc"]
    import sys
    try:
        import concourse.bacc as bacc
    except ImportError:
        sys.path.append("/opt/trn_rl_repo")
        import concourse.bacc as bacc
    import concourse.mybir as mybir
    from concourse.tile import TileContext

    A = mybir.AluOpType
    f32 = mybir.dt.float32
    nc = bacc.Bacc()
    cb_d = nc.dram_tensor("cb", [CH * M * P, F], f32, kind="ExternalInput")
    z0re_d = nc.dram_tensor("z0re", [CH * P, F], f32, kind="ExternalInput")
    z0im_d = nc.dram_tensor("z0im", [CH * P, F], f32, kind="ExternalInput")
    rb_d = nc.dram_tensor("rb", [CH * P, F], f32, kind="ExternalInput")
    ore_d = nc.dram_tensor("ore", [CH * P, F], f32, kind="ExternalOutput")
    oim_d = nc.dram_tensor("oim", [CH * P, F], f32, kind="ExternalOutput")

    with TileContext(nc) as tc:
        with tc.tile_pool(name="pool", bufs=1) as pool:
            def T(name):
                return pool.tile([P, F], f32, name=name)

            cb = [T("cb%d" % k) for k in range(M)]
            names = ("zre zim hre him qre qim Sre Sim Rb t1 t2 t3 t4 d2 rr "
                     "rr2 mx mm wre wim dnre dnim sq1 sq2").split()
            (zre, zim, hre, him, qre, qim, Sre, Sim, Rb, t1, t2, t3, t4, d2,
             rr, rr2, mx, mm, wre, wim, dnre, dnim, sq1, sq2) = (
                T(n) for n in names)
            v = nc.vector
            sc = nc.scalar

            def V(t):
                return t[:].rearrange("p (g i) -> p g i", i=M)

            for ch in range(CH):
                for k in range(M):
                    r0 = (ch * M + k) * P
                    nc.sync.dma_start(out=cb[k][:], in_=cb_d[r0:r0 + P])
                nc.sync.dma_start(out=zre[:], in_=z0re_d[ch * P:(ch + 1) * P])
                nc.sync.dma_start(out=zim[:], in_=z0im_d[ch * P:(ch + 1) * P])
                nc.sync.dma_start(out=Rb[:], in_=rb_d[ch * P:(ch + 1) * P])

                for _ in range(NITER):
                    # Horner for p (h) and p' (q).  k=0: h=z-c0, q=1.
                    # k=1 collapses to q = z + h.
                    v.tensor_sub(hre[:], zre[:], cb[0][:])
                    sc.copy(him[:], zim[:])
                    v.tensor_add(qre[:], zre[:], hre[:])
                    v.tensor_add(qim[:], zim[:], him[:])
                    # k=1 h-update: h = h*z - c1
                    v.tensor_mul(t1[:], hre[:], zre[:])
                    v.tensor_mul(t2[:], him[:], zim[:])
                    v.tensor_mul(t3[:], hre[:], zim[:])
                    v.tensor_mul(t4[:], him[:], zre[:])
                    v.tensor_sub(hre[:], t1[:], t2[:])
                    v.tensor_sub(hre[:], hre[:], cb[1][:])
                    v.tensor_add(him[:], t3[:], t4[:])
                    for k in range(2, M):
                        # q = q*z + h  (uses h from step k-1)
                        v.tensor_mul(t1[:], qre[:], zre[:])
                        v.tensor_mul(t2[:], qim[:], zim[:])
                        v.tensor_mul(t3[:], qre[:], zim[:])
                        v.tensor_mul(t4[:], qim[:], zre[:])
                        v.tensor_sub(qre[:], t1[:], t2[:])
                        v.tensor_add(qre[:], qre[:], hre[:])
                        v.tensor_add(qim[:], t3[:], t4[:])
                        v.tensor_add(qim[:], qim[:], him[:])
                        # h = h*z - c[k]
                        v.tensor_mul(t1[:], hre[:], zre[:])
                        v.tensor_mul(t2[:], him[:], zim[:])
                        v.tensor_mul(t3[:], hre[:], zim[:])
                        v.tensor_mul(t4[:], him[:], zre[:])
                        v.tensor_sub(hre[:], t1[:], t2[:])
                        v.tensor_sub(hre[:], hre[:], cb[k][:])
                        v.tensor_add(him[:], t3[:], t4[:])

                    # Pairwise repulsion S = sum_{j!=i} 1/(z_i - z_j),
                    # mirrored shifts: s and M-s terms share one reciprocal.
                    for s in range(1, 17):
                        zreV, zimV = V(zre), V(zim)
                        dreV, dimV = V(t3), V(t4)
                        v.tensor_sub(dreV[:, :, 0:M - s], zreV[:, :, 0:M - s],
                                     zreV[:, :, s:M])
                        v.tensor_sub(dreV[:, :, M - s:M], zreV[:, :, M - s:M],
                                     zreV[:, :, 0:s])
                        v.tensor_sub(dimV[:, :, 0:M - s], zimV[:, :, 0:M - s],
                                     zimV[:, :, s:M])
                        v.tensor_sub(dimV[:, :, M - s:M], zimV[:, :, M - s:M],
                                     zimV[:, :, 0:s])
                        sc.square(sq1[:], t3[:])
                        sc.square(sq2[:], t4[:])
                        v.scalar_tensor_tensor(d2[:], sq1[:], SEPS, sq2[:],
                                               A.add, A.add)
                        v.reciprocal_approx_fast(rr[:], d2[:])
                        v.tensor_mul(t1[:], t3[:], rr[:])   # tre
                        v.tensor_mul(t2[:], t4[:], rr[:])   # tim
                        if s == 1:
                            sc.copy(Sre[:], t1[:])
                            sc.mul(Sim[:], t2[:], -1.0)
                        else:
                            v.tensor_add(Sre[:], Sre[:], t1[:])
                            v.tensor_sub(Sim[:], Sim[:], t2[:])
                        if s < 16:
                            SreV, SimV, treV, timV = V(Sre), V(Sim), V(t1), V(t2)
                            v.tensor_sub(SreV[:, :, s:M], SreV[:, :, s:M],
                                         treV[:, :, 0:M - s])
                            v.tensor_sub(SreV[:, :, 0:s], SreV[:, :, 0:s],
                                         treV[:, :, M - s:M])
                            v.tensor_add(SimV[:, :, s:M], SimV[:, :, s:M],
                                         timV[:, :, 0:M - s])
                            v.tensor_add(SimV[:, :, 0:s], SimV[:, :, 0:s],
                                         timV[:, :, M - s:M])

                    # joint scale-normalization of (p, p') by component max
                    # (|x| = max(-x, x); abs_max TT op not supported by codegen)
                    v.scalar_tensor_tensor(mx[:], hre[:], -1.0, hre[:],
                                           A.mult, A.max)
                    v.scalar_tensor_tensor(t1[:], him[:], -1.0, him[:],
                                           A.mult, A.max)
                    v.tensor_tensor(mx[:], mx[:], t1[:], op=A.max)
                    v.scalar_tensor_tensor(t1[:], qre[:], -1.0, qre[:],
                                           A.mult, A.max)
                    v.tensor_tensor(mx[:], mx[:], t1[:], op=A.max)
                    v.scalar_tensor_tensor(t1[:], qim[:], -1.0, qim[:],
                                           A.mult, A.max)
                    v.tensor_tensor(mx[:], mx[:], t1[:], op=A.max)
                    v.tensor_scalar_add(mx[:], mx[:], EPS)
                    v.reciprocal_approx_fast(mm[:], mx[:])
                    v.tensor_mul(hre[:], hre[:], mm[:])
                    v.tensor_mul(him[:], him[:], mm[:])
                    v.tensor_mul(qre[:], qre[:], mm[:])
                    v.tensor_mul(qim[:], qim[:], mm[:])
                    # den = p'n - pn*S
                    v.tensor_mul(t1[:], hre[:], Sre[:])
                    v.tensor_mul(t2[:], him[:], Sim[:])
                    v.tensor_mul(t3[:], hre[:], Sim[:])
                    v.tensor_mul(t4[:], him[:], Sre[:])
                    v.tensor_sub(dnre[:], qre[:], t1[:])
                    v.tensor_add(dnre[:], dnre[:], t2[:])
                    v.tensor_sub(dnim[:], qim[:], t3[:])
                    v.tensor_sub(dnim[:], dnim[:], t4[:])
                    sc.square(sq1[:], dnre[:])
                    sc.square(sq2[:], dnim[:])
                    v.scalar_tensor_tensor(d2[:], sq1[:], EPS, sq2[:],
                                           A.add, A.add)
                    v.reciprocal_approx_fast(rr[:], d2[:])
                    # w = pn * conj(den) / |den|^2
                    v.tensor_mul(t1[:], hre[:], dnre[:])
                    v.tensor_mul(t2[:], him[:], dnim[:])
                    v.tensor_add(t1[:], t1[:], t2[:])
                    v.tensor_mul(t3[:], him[:], dnre[:])
                    v.tensor_mul(t4[:], hre[:], dnim[:])
                    v.tensor_sub(t3[:], t3[:], t4[:])
                    v.tensor_mul(wre[:], t1[:], rr[:])
                    v.tensor_mul(wim[:], t3[:], rr[:])
                    # cap |w| <= 1
                    sc.square(sq1[:], wre[:])
                    sc.square(sq2[:], wim[:])
                    v.scalar_tensor_tensor(d2[:], sq1[:], EPS, sq2[:],
                                           A.add, A.add)
                    v.reciprocal_approx_fast(rr[:], d2[:])
                    sc.sqrt(rr2[:], rr[:])
                    v.tensor_scalar_min(mm[:], rr2[:], 1.0)
                    v.tensor_mul(wre[:], wre[:], mm[:])
                    v.tensor_mul(wim[:], wim[:], mm[:])
                    v.tensor_sub(zre[:], zre[:], wre[:])
                    v.tensor_sub(zim[:], zim[:], wim[:])
                    # clamp |z| <= R
                    sc.square(sq1[:], zre[:])
                    sc.square(sq2[:], zim[:])
                    v.scalar_tensor_tensor(d2[:], sq1[:], EPS, sq2[:],
                                           A.add, A.add)
                    v.reciprocal_approx_fast(rr[:], d2[:])
                    sc.sqrt(rr2[:], rr[:])
                    v.tensor_mul(mm[:], rr2[:], Rb[:])
                    v.tensor_scalar_min(mm[:], mm[:], 1.0)
                    v.tensor_mul(zre[:], zre[:], mm[:])
                    v.tensor_mul(zim[:], zim[:], mm[:])

                nc.sync.dma_start(out=ore_d[ch * P:(ch + 1) * P], in_=zre[:])
                nc.sync.dma_start(out=oim_d[ch * P:(ch + 1) * P], in_=zim[:])

    nc.compile()
    _CACHE["nc"] = nc
    return nc


def _pack_plane(x):
    """(ROWS_CORE, M) -> (CH*P, F) with plane[p, g*M+i] = x[ch*G*P + g*P + p, i]."""
    return np.ascontiguousarray(
        x.reshape(CH, G, P, M).transpose(0, 2, 1, 3).reshape(CH * P, F))


def _unpack_plane(y):
    return y.reshape(CH, P, G, M).transpose(0, 2, 1, 3).reshape(ROWS_CORE, M)


def _pack_cb(c):
    """(ROWS_CORE, M) coeffs -> (CH*M*P, F), coeff k broadcast over the root axis."""
    cc = c.reshape(CH, G, P, M).transpose(0, 3, 2, 1)          # [ch, k, p, g]
    cc = np.broadcast_to(cc[..., None], (CH, M, P, G, M))
    return np.ascontiguousarray(cc.reshape(CH * M * P, F))


def _symmetrize_sort(z):
    """Pair conjugate roots exactly, zero near-real imags, canonical sort."""
    re = z.real.astype(np.float32)
    im = z.imag.astype(np.float32)
    tol = np.float32(1e-5)
    near_real = np.abs(im) <= tol * (1.0 + np.abs(re))
    K = np.where(near_real, 0.0, np.sign(im)).astype(np.float32)
    aim = np.where(near_real, np.float32(0.0), np.abs(im))
    idx = np.lexsort((aim, re, K), axis=-1)
    reS = np.take_along_axis(re, idx, -1)
    imS = np.take_along_axis(im, idx, -1)
    KS = np.take_along_axis(K, idx, -1)
    nN = (KS == -1).sum(-1, keepdims=True)
    nR = (KS == 0).sum(-1, keepdims=True)
    nP = (KS == 1).sum(-1, keepdims=True)
    ok = (nN == nP)
    j = np.arange(M)[None, :]
    isN = j < nN
    isP = j >= (nN + nR)
    partner = np.where(isN, j + nN + nR, np.where(isP, j - nN - nR, j))
    partner = np.clip(partner, 0, M - 1)
    reP = np.take_along_axis(reS, partner, -1)
    imP = np.take_along_axis(imS, partner, -1)
    mre = np.float32(0.5) * (reS + reP)
    sgn = np.where(isP, np.float32(1.0), np.where(isN, np.float32(-1.0),
                                                  np.float32(0.0)))
    mag = np.float32(0.5) * np.abs(imS - imP)
    mim = sgn * mag
    fre = np.where(ok, mre, reS).astype(np.float32)
    fim = np.where(ok, mim, imS).astype(np.float32)
    idx2 = np.lexsort((fim, fre), axis=-1)
    fre = np.take_along_axis(fre, idx2, -1)
    fim = np.take_along_axis(fim, idx2, -1)
    return (fre + 1j * fim).astype(np.complex64)


def kernel(a, _trace=False):
    import sys
    try:
        from concourse.bass_utils import run_bass_kernel_spmd
    except ImportError:
        sys.path.append("/opt/trn_rl_repo")
        from concourse.bass_utils import run_bass_kernel_spmd

    nc = _get_nc()
    a = np.asarray(a, dtype=np.float32)
    c = (-(a[:, 1:] / a[:, :1])).astype(np.float32)
    r = np.clip(np.abs(c[:, -1:]) ** (1.0 / M), 0.5, 2.0).astype(np.float32)
    ang = ((2.0 * np.pi / M) * np.arange(M, dtype=np.float32)
           + np.float32(0.4)).astype(np.float32)
    z0re = (r * np.cos(ang)[None, :]).astype(np.float32)
    z0im = (r * np.sin(ang)[None, :]).astype(np.float32)
    R = np.minimum(1.0 + np.max(np.abs(c), axis=1, keepdims=True),
                   11.0).astype(np.float32)
    Rb = np.broadcast_to(R, (B, M)).astype(np.float32)

    in_maps = []
    for core in range(NCORES):
        sl = slice(core * ROWS_CORE, (core + 1) * ROWS_CORE)
        in_maps.append({
            "cb": _pack_cb(c[sl]),
            "z0re": _pack_plane(z0re[sl]),
            "z0im": _pack_plane(z0im[sl]),
            "rb": _pack_plane(Rb[sl]),
        })
    out = run_bass_kernel_spmd(nc, in_maps, list(range(NCORES)), trace=_trace)
    if _trace:
        _CACHE["exec_time_ns"] = out.exec_time_ns
    res = out.results
    cores = []
    for core in range(NCORES):
        zre = _unpack_plane(res[core]["ore"])
        zim = _unpack_plane(res[core]["oim"])
        cores.append(zre + 1j * zim)
    z = np.concatenate(cores, axis=0).astype(np.complex64)
    return _symmetrize_sort(z)


# revision 17
# speedup vs baseline: 1.1034x; 1.1034x over previous
"""PolynomialToRoots on 8 Trainium2 NeuronCores.

Finds all 32 roots of 32768 degree-32 real polynomials (companion-matrix
eigenvalues) with a batched, guarded Ehrlich-Aberth iteration run entirely
on-device, data-parallel over the batch axis (4096 polynomials per core).

Output roots are conjugate-symmetrized and canonically sorted per row
(eigenvalue order of LAPACK geev is implementation-defined, so any
order-invariant comparison must sort; we return a deterministic canonical
order).
"""
import numpy as np

M = 32                     # polynomial degree / roots per row
P = 128                    # SBUF partitions
G = 16                     # row-groups per chunk
F = G * M                  # free dim: 512
CH = 2                     # chunks per core
ROWS_CORE = CH * G * P     # 4096
NCORES = 8
B = ROWS_CORE * NCORES     # 32768
NITER = 20
EPS = 1e-36
SEPS = 1e-30

_CACHE = {}


def _get_nc():
    if "nc" in _CACHE:
        return _CACHE["nc"]
    import sys
    try:
        import concourse.bacc as bacc
    except ImportError:
        sys.path.append("/opt/trn_rl_repo")
        import concourse.bacc as bacc
    import concourse.mybir as mybir
    from concourse.tile import TileContext

    A = mybir.AluOpType
    f32 = mybir.dt.float32
    nc = bacc.Bacc()
    cb_d = nc.dram_tensor("cb", [CH * M * P, F], f32, kind="ExternalInput")
    z0re_d = nc.dram_tensor("z0re", [CH * P, F], f32, kind="ExternalInput")
    z0im_d = nc.dram_tensor("z0im", [CH * P, F], f32, kind="ExternalInput")
    rb_d = nc.dram_tensor("rb", [CH * P, F], f32, kind="ExternalInput")
    ore_d = nc.dram_tensor("ore", [CH * P, F], f32, kind="ExternalOutput")
    oim_d = nc.dram_tensor("oim", [CH * P, F], f32, kind="ExternalOutput")

    with TileContext(nc) as tc:
        with tc.tile_pool(name="pool", bufs=1) as pool:
            def T(name):
                return pool.tile([P, F], f32, name=name)

            cb = [T("cb%d" % k) for k in range(M)]
            names = ("zre zim hre him qre qim Sre Sim Rb t1 t2 t3 t4 d2 rr "
                     "rr2 mx mm wre wim dnre dnim sq1 sq2").split()
            (zre, zim, hre, him, qre, qim, Sre, Sim, Rb, t1, t2, t3, t4, d2,
             rr, rr2, mx, mm, wre, wim, dnre, dnim, sq1, sq2) = (
                T(n) for n in names)
            # extended (doubled) planes: [p, g, 0:32] and [p, g, 32:64] both
            # hold z, so shifted reads z[i+s] and mirror writes S[i+s] need no
            # circular wrap-split.
            zxre = pool.tile([P, 2 * F], f32, name="zxre")
            zxim = pool.tile([P, 2 * F], f32, name="zxim")
            Sxre = pool.tile([P, 2 * F], f32, name="Sxre")
            Sxim = pool.tile([P, 2 * F], f32, name="Sxim")
            v = nc.vector
            sc = nc.scalar

            def V(t):
                return t[:].rearrange("p (g i) -> p g i", i=M)

            def VX(t):
                return t[:].rearrange("p (g i) -> p g i", i=2 * M)

            for ch in range(CH):
                for k in range(M):
                    r0 = (ch * M + k) * P
                    nc.sync.dma_start(out=cb[k][:], in_=cb_d[r0:r0 + P])
                nc.sync.dma_start(out=zre[:], in_=z0re_d[ch * P:(ch + 1) * P])
                nc.sync.dma_start(out=zim[:], in_=z0im_d[ch * P:(ch + 1) * P])
                nc.sync.dma_start(out=Rb[:], in_=rb_d[ch * P:(ch + 1) * P])
                # zx = [z, z] per group (ACT engine; DVE stays busy)
                sc.copy(VX(zxre)[:, :, 0:M], V(zre))
                sc.copy(VX(zxre)[:, :, M:2 * M], V(zre))
                sc.copy(VX(zxim)[:, :, 0:M], V(zim))
                sc.copy(VX(zxim)[:, :, M:2 * M], V(zim))
                # exact zero init (uninitialized SBUF may hold NaN; x*0 would
                # keep it NaN, z-z cannot)
                for half in (slice(0, M), slice(M, 2 * M)):
                    v.tensor_sub(VX(Sxre)[:, :, half], V(zre), V(zre))
                    v.tensor_sub(VX(Sxim)[:, :, half], V(zre), V(zre))

                for it in range(NITER):
                    # Horner for p (h) and p' (q).  k=0: h=z-c0, q=1.
                    # k=1 collapses to q = z + h.
                    v.tensor_sub(hre[:], zre[:], cb[0][:])
                    sc.copy(him[:], zim[:])
                    v.tensor_add(qre[:], zre[:], hre[:])
                    v.tensor_add(qim[:], zim[:], him[:])
                    # k=1 h-update: h = h*z - c1
                    v.tensor_mul(t1[:], hre[:], zre[:])
                    v.tensor_mul(t2[:], him[:], zim[:])
                    v.tensor_mul(t3[:], hre[:], zim[:])
                    v.tensor_mul(t4[:], him[:], zre[:])
                    v.tensor_sub(hre[:], t1[:], t2[:])
                    v.tensor_sub(hre[:], hre[:], cb[1][:])
                    v.tensor_add(him[:], t3[:], t4[:])
                    for k in range(2, M):
                        # q = q*z + h  (uses h from step k-1)
                        v.tensor_mul(t1[:], qre[:], zre[:])
                        v.tensor_mul(t2[:], qim[:], zim[:])
                        v.tensor_mul(t3[:], qre[:], zim[:])
                        v.tensor_mul(t4[:], qim[:], zre[:])
                        v.tensor_sub(qre[:], t1[:], t2[:])
                        v.tensor_add(qre[:], qre[:], hre[:])
                        v.tensor_add(qim[:], t3[:], t4[:])
                        v.tensor_add(qim[:], qim[:], him[:])
                        # h = h*z - c[k]
                        v.tensor_mul(t1[:], hre[:], zre[:])
                        v.tensor_mul(t2[:], him[:], zim[:])
                        v.tensor_mul(t3[:], hre[:], zim[:])
                        v.tensor_mul(t4[:], him[:], zre[:])
                        v.tensor_sub(hre[:], t1[:], t2[:])
                        v.tensor_sub(hre[:], hre[:], cb[k][:])
                        v.tensor_add(him[:], t3[:], t4[:])

                    # Pairwise repulsion S = sum_{j!=i} 1/(z_i - z_j),
                    # mirrored shifts: s and M-s terms share one reciprocal.
                    # Shifted reads come from doubled zx; mirror writes go to
                    # extended Sx (zeroed on ACT), folded into S afterwards.
                    sc.mul(Sxre[:], Sxre[:], 0.0)
                    sc.mul(Sxim[:], Sxim[:], 0.0)
                    for s in range(1, 17):
                        v.tensor_sub(V(t3), V(zre), VX(zxre)[:, :, s:s + M])
                        v.tensor_sub(V(t4), V(zim), VX(zxim)[:, :, s:s + M])
                        sc.square(sq1[:], t3[:])
                        sc.square(sq2[:], t4[:])
                        v.scalar_tensor_tensor(d2[:], sq1[:], SEPS, sq2[:],
                                               A.add, A.add)
                        v.reciprocal_approx_fast(rr[:], d2[:])
                        v.tensor_mul(t1[:], t3[:], rr[:])   # tre
                        v.tensor_mul(t2[:], t4[:], rr[:])   # tim
                        if s == 1:
                            sc.copy(Sre[:], t1[:])
                            sc.mul(Sim[:], t2[:], -1.0)
                        else:
                            v.tensor_add(Sre[:], Sre[:], t1[:])
                            v.tensor_sub(Sim[:], Sim[:], t2[:])
                        if s < 16:
                            v.tensor_sub(VX(Sxre)[:, :, s:s + M],
                                         VX(Sxre)[:, :, s:s + M], V(t1))
                            v.tensor_add(VX(Sxim)[:, :, s:s + M],
                                         VX(Sxim)[:, :, s:s + M], V(t2))
                    v.tensor_add(V(Sre), V(Sre), VX(Sxre)[:, :, 0:M])
                    v.tensor_add(V(Sre), V(Sre), VX(Sxre)[:, :, M:2 * M])
                    v.tensor_add(V(Sim), V(Sim), VX(Sxim)[:, :, 0:M])
                    v.tensor_add(V(Sim), V(Sim), VX(Sxim)[:, :, M:2 * M])

                    # joint scale-normalization of (p, p') by component max
                    # (|x| = max(-x, x); abs_max TT op not supported by codegen)
                    v.scalar_tensor_tensor(mx[:], hre[:], -1.0, hre[:],
                                           A.mult, A.max)
                    v.scalar_tensor_tensor(t1[:], him[:], -1.0, him[:],
                                           A.mult, A.max)
                    v.tensor_tensor(mx[:], mx[:], t1[:], op=A.max)
                    v.scalar_tensor_tensor(t1[:], qre[:], -1.0, qre[:],
                                           A.mult, A.max)
                    v.tensor_tensor(mx[:], mx[:], t1[:], op=A.max)
                    v.scalar_tensor_tensor(t1[:], qim[:], -1.0, qim[:],
                                           A.mult, A.max)
                    v.tensor_tensor(mx[:], mx[:], t1[:], op=A.max)
                    v.tensor_scalar_add(mx[:], mx[:], EPS)
                    v.reciprocal_approx_fast(mm[:], mx[:])
                    v.tensor_mul(hre[:], hre[:], mm[:])
                    v.tensor_mul(him[:], him[:], mm[:])
                    v.tensor_mul(qre[:], qre[:], mm[:])
                    v.tensor_mul(qim[:], qim[:], mm[:])
                    # den = p'n - pn*S
                    v.tensor_mul(t1[:], hre[:], Sre[:])
                    v.tensor_mul(t2[:], him[:], Sim[:])
                    v.tensor_mul(t3[:], hre[:], Sim[:])
                    v.tensor_mul(t4[:], him[:], Sre[:])
                    v.tensor_sub(dnre[:], qre[:], t1[:])
                    v.tensor_add(dnre[:], dnre[:], t2[:])
                    v.tensor_sub(dnim[:], qim[:], t3[:])
                    v.tensor_sub(dnim[:], dnim[:], t4[:])
                    sc.square(sq1[:], dnre[:])
                    sc.square(sq2[:], dnim[:])
                    v.scalar_tensor_tensor(d2[:], sq1[:], EPS, sq2[:],
                                           A.add, A.add)
                    v.reciprocal_approx_fast(rr[:], d2[:])
                    # w = pn * conj(den) / |den|^2
                    v.tensor_mul(t1[:], hre[:], dnre[:])
                    v.tensor_mul(t2[:], him[:], dnim[:])
                    v.tensor_add(t1[:], t1[:], t2[:])
                    v.tensor_mul(t3[:], him[:], dnre[:])
                    v.tensor_mul(t4[:], hre[:], dnim[:])
                    v.tensor_sub(t3[:], t3[:], t4[:])
                    v.tensor_mul(wre[:], t1[:], rr[:])
                    v.tensor_mul(wim[:], t3[:], rr[:])
                    # cap |w| <= 1
                    sc.square(sq1[:], wre[:])
                    sc.square(sq2[:], wim[:])
                    v.scalar_tensor_tensor(d2[:], sq1[:], EPS, sq2[:],
                                           A.add, A.add)
                    v.reciprocal_approx_fast(rr[:], d2[:])
                    sc.sqrt(rr2[:], rr[:])
                    v.tensor_scalar_min(mm[:], rr2[:], 1.0)
                    v.tensor_mul(wre[:], wre[:], mm[:])
                    v.tensor_mul(wim[:], wim[:], mm[:])
                    v.tensor_sub(zre[:], zre[:], wre[:])
                    v.tensor_sub(zim[:], zim[:], wim[:])
                    # clamp |z| <= R
                    sc.square(sq1[:], zre[:])
                    sc.square(sq2[:], zim[:])
                    v.scalar_tensor_tensor(d2[:], sq1[:], EPS, sq2[:],
                                           A.add, A.add)
                    v.reciprocal_approx_fast(rr[:], d2[:])
                    sc.sqrt(rr2[:], rr[:])
                    v.tensor_mul(mm[:], rr2[:], Rb[:])
                    v.tensor_scalar_min(mm[:], mm[:], 1.0)
                    v.tensor_mul(zre[:], zre[:], mm[:])
                    v.tensor_mul(zim[:], zim[:], mm[:])
                    if it < NITER - 1:
                        sc.copy(VX(zxre)[:, :, 0:M], V(zre))
                        sc.copy(VX(zxre)[:, :, M:2 * M], V(zre))
                        sc.copy(VX(zxim)[:, :, 0:M], V(zim))
                        sc.copy(VX(zxim)[:, :, M:2 * M], V(zim))

                nc.sync.dma_start(out=ore_d[ch * P:(ch + 1) * P], in_=zre[:])
                nc.sync.dma_start(out=oim_d[ch * P:(ch + 1) * P], in_=zim[:])

    nc.compile()
    _CACHE["nc"] = nc
    return nc


def _pack_plane(x):
    """(ROWS_CORE, M) -> (CH*P, F) with plane[p, g*M+i] = x[ch*G*P + g*P + p, i]."""
    return np.ascontiguousarray(
        x.reshape(CH, G, P, M).transpose(0, 2, 1, 3).reshape(CH * P, F))


def _unpack_plane(y):
    return y.reshape(CH, P, G, M).transpose(0, 2, 1, 3).reshape(ROWS_CORE, M)


def _pack_cb(c):
    """(ROWS_CORE, M) coeffs -> (CH*M*P, F), coeff k broadcast over the root axis."""
    cc = c.reshape(CH, G, P, M).transpose(0, 3, 2, 1)          # [ch, k, p, g]
    cc = np.broadcast_to(cc[..., None], (CH, M, P, G, M))
    return np.ascontiguousarray(cc.reshape(CH * M * P, F))


def _symmetrize_sort(z):
    """Pair conjugate roots exactly, zero near-real imags, canonical sort."""
    re = z.real.astype(np.float32)
    im = z.imag.astype(np.float32)
    tol = np.float32(1e-5)
    near_real = np.abs(im) <= tol * (1.0 + np.abs(re))
    K = np.where(near_real, 0.0, np.sign(im)).astype(np.float32)
    aim = np.where(near_real, np.float32(0.0), np.abs(im))
    idx = np.lexsort((aim, re, K), axis=-1)
    reS = np.take_along_axis(re, idx, -1)
    imS = np.take_along_axis(im, idx, -1)
    KS = np.take_along_axis(K, idx, -1)
    nN = (KS == -1).sum(-1, keepdims=True)
    nR = (KS == 0).sum(-1, keepdims=True)
    nP = (KS == 1).sum(-1, keepdims=True)
    ok = (nN == nP)
    j = np.arange(M)[None, :]
    isN = j < nN
    isP = j >= (nN + nR)
    partner = np.where(isN, j + nN + nR, np.where(isP, j - nN - nR, j))
    partner = np.clip(partner, 0, M - 1)
    reP = np.take_along_axis(reS, partner, -1)
    imP = np.take_along_axis(imS, partner, -1)
    mre = np.float32(0.5) * (reS + reP)
    sgn = np.where(isP, np.float32(1.0), np.where(isN, np.float32(-1.0),
                                                  np.float32(0.0)))
    mag = np.float32(0.5) * np.abs(imS - imP)
    mim = sgn * mag
    fre = np.where(ok, mre, reS).astype(np.float32)
    fim = np.where(ok, mim, imS).astype(np.float32)
    idx2 = np.lexsort((fim, fre), axis=-1)
    fre = np.take_along_axis(fre, idx2, -1)
    fim = np.take_along_axis(fim, idx2, -1)
    return (fre + 1j * fim).astype(np.complex64)


def kernel(a, _trace=False):
    import sys
    try:
        from concourse.bass_utils import run_bass_kernel_spmd
    except ImportError:
        sys.path.append("/opt/trn_rl_repo")
        from concourse.bass_utils import run_bass_kernel_spmd

    nc = _get_nc()
    a = np.asarray(a, dtype=np.float32)
    c = (-(a[:, 1:] / a[:, :1])).astype(np.float32)
    r = np.clip(np.abs(c[:, -1:]) ** (1.0 / M), 0.5, 2.0).astype(np.float32)
    ang = ((2.0 * np.pi / M) * np.arange(M, dtype=np.float32)
           + np.float32(0.4)).astype(np.float32)
    z0re = (r * np.cos(ang)[None, :]).astype(np.float32)
    z0im = (r * np.sin(ang)[None, :]).astype(np.float32)
    R = np.minimum(1.0 + np.max(np.abs(c), axis=1, keepdims=True),
                   11.0).astype(np.float32)
    Rb = np.broadcast_to(R, (B, M)).astype(np.float32)

    in_maps = []
    for core in range(NCORES):
        sl = slice(core * ROWS_CORE, (core + 1) * ROWS_CORE)
        in_maps.append({
            "cb": _pack_cb(c[sl]),
            "z0re": _pack_plane(z0re[sl]),
            "z0im": _pack_plane(z0im[sl]),
            "rb": _pack_plane(Rb[sl]),
        })
    out = run_bass_kernel_spmd(nc, in_maps, list(range(NCORES)), trace=_trace)
    if _trace:
        _CACHE["exec_time_ns"] = out.exec_time_ns
    res = out.results
    cores = []
    for core in range(NCORES):
        zre = _unpack_plane(res[core]["ore"])
        zim = _unpack_plane(res[core]["oim"])
        cores.append(zre + 1j * zim)
    z = np.concatenate(cores, axis=0).astype(np.complex64)
    return _symmetrize_sort(z)


# revision 18
# speedup vs baseline: 1.1608x; 1.0520x over previous
"""PolynomialToRoots on 8 Trainium2 NeuronCores.

Finds all 32 roots of 32768 degree-32 real polynomials (companion-matrix
eigenvalues) with a batched, guarded Ehrlich-Aberth iteration run entirely
on-device, data-parallel over the batch axis (4096 polynomials per core).

Output roots are conjugate-symmetrized and canonically sorted per row
(eigenvalue order of LAPACK geev is implementation-defined, so any
order-invariant comparison must sort; we return a deterministic canonical
order).
"""
import numpy as np

M = 32                     # polynomial degree / roots per row
P = 128                    # SBUF partitions
G = 16                     # row-groups per chunk
F = G * M                  # free dim: 512
CH = 2                     # chunks per core
ROWS_CORE = CH * G * P     # 4096
NCORES = 8
B = ROWS_CORE * NCORES     # 32768
NITER = 19
EPS = 1e-36
SEPS = 1e-30

_CACHE = {}


def _get_nc():
    if "nc" in _CACHE:
        return _CACHE["nc"]
    import sys
    try:
        import concourse.bacc as bacc
    except ImportError:
        sys.path.append("/opt/trn_rl_repo")
        import concourse.bacc as bacc
    import concourse.mybir as mybir
    from concourse.tile import TileContext

    A = mybir.AluOpType
    f32 = mybir.dt.float32
    nc = bacc.Bacc()
    cb_d = nc.dram_tensor("cb", [CH * M * P, F], f32, kind="ExternalInput")
    z0re_d = nc.dram_tensor("z0re", [CH * P, F], f32, kind="ExternalInput")
    z0im_d = nc.dram_tensor("z0im", [CH * P, F], f32, kind="ExternalInput")
    rb_d = nc.dram_tensor("rb", [CH * P, F], f32, kind="ExternalInput")
    ore_d = nc.dram_tensor("ore", [CH * P, F], f32, kind="ExternalOutput")
    oim_d = nc.dram_tensor("oim", [CH * P, F], f32, kind="ExternalOutput")

    with TileContext(nc) as tc:
        with tc.tile_pool(name="pool", bufs=1) as pool:
            def T(name):
                return pool.tile([P, F], f32, name=name)

            cb = [T("cb%d" % k) for k in range(M)]
            names = ("zre zim hre him qre qim Sre Sim Rb t1 t2 t3 t4 d2 rr "
                     "rr2 mx mm wre wim dnre dnim sq1 sq2").split()
            (zre, zim, hre, him, qre, qim, Sre, Sim, Rb, t1, t2, t3, t4, d2,
             rr, rr2, mx, mm, wre, wim, dnre, dnim, sq1, sq2) = (
                T(n) for n in names)
            # extended (doubled) planes: [p, g, 0:32] and [p, g, 32:64] both
            # hold z, so shifted reads z[i+s] and mirror writes S[i+s] need no
            # circular wrap-split.
            zxre = pool.tile([P, 2 * F], f32, name="zxre")
            zxim = pool.tile([P, 2 * F], f32, name="zxim")
            Sxre = pool.tile([P, 2 * F], f32, name="Sxre")
            Sxim = pool.tile([P, 2 * F], f32, name="Sxim")
            v = nc.vector
            sc = nc.scalar

            def V(t):
                return t[:].rearrange("p (g i) -> p g i", i=M)

            def VX(t):
                return t[:].rearrange("p (g i) -> p g i", i=2 * M)

            for ch in range(CH):
                for k in range(M):
                    r0 = (ch * M + k) * P
                    nc.sync.dma_start(out=cb[k][:], in_=cb_d[r0:r0 + P])
                nc.sync.dma_start(out=zre[:], in_=z0re_d[ch * P:(ch + 1) * P])
                nc.sync.dma_start(out=zim[:], in_=z0im_d[ch * P:(ch + 1) * P])
                nc.sync.dma_start(out=Rb[:], in_=rb_d[ch * P:(ch + 1) * P])
                # zx = [z, z] per group (ACT engine; DVE stays busy)
                sc.copy(VX(zxre)[:, :, 0:M], V(zre))
                sc.copy(VX(zxre)[:, :, M:2 * M], V(zre))
                sc.copy(VX(zxim)[:, :, 0:M], V(zim))
                sc.copy(VX(zxim)[:, :, M:2 * M], V(zim))
                # exact zero init (uninitialized SBUF may hold NaN; x*0 would
                # keep it NaN, z-z cannot)
                for half in (slice(0, M), slice(M, 2 * M)):
                    v.tensor_sub(VX(Sxre)[:, :, half], V(zre), V(zre))
                    v.tensor_sub(VX(Sxim)[:, :, half], V(zre), V(zre))

                for it in range(NITER):
                    # Horner for p (h) and p' (q).  k=0: h=z-c0, q=1.
                    # k=1 collapses to q = z + h.
                    v.tensor_sub(hre[:], zre[:], cb[0][:])
                    sc.copy(him[:], zim[:])
                    v.tensor_add(qre[:], zre[:], hre[:])
                    v.tensor_add(qim[:], zim[:], him[:])
                    # k=1 h-update: h = h*z - c1
                    v.tensor_mul(t1[:], hre[:], zre[:])
                    v.tensor_mul(t2[:], him[:], zim[:])
                    v.tensor_mul(t3[:], hre[:], zim[:])
                    v.tensor_mul(t4[:], him[:], zre[:])
                    v.tensor_sub(hre[:], t1[:], t2[:])
                    v.tensor_sub(hre[:], hre[:], cb[1][:])
                    v.tensor_add(him[:], t3[:], t4[:])
                    for k in range(2, M):
                        # q = q*z + h  (uses h from step k-1)
                        v.tensor_mul(t1[:], qre[:], zre[:])
                        v.tensor_mul(t2[:], qim[:], zim[:])
                        v.tensor_mul(t3[:], qre[:], zim[:])
                        v.tensor_mul(t4[:], qim[:], zre[:])
                        v.tensor_sub(qre[:], t1[:], t2[:])
                        v.tensor_add(qre[:], qre[:], hre[:])
                        v.tensor_add(qim[:], t3[:], t4[:])
                        v.tensor_add(qim[:], qim[:], him[:])
                        # h = h*z - c[k]
                        v.tensor_mul(t1[:], hre[:], zre[:])
                        v.tensor_mul(t2[:], him[:], zim[:])
                        v.tensor_mul(t3[:], hre[:], zim[:])
                        v.tensor_mul(t4[:], him[:], zre[:])
                        v.tensor_sub(hre[:], t1[:], t2[:])
                        v.tensor_sub(hre[:], hre[:], cb[k][:])
                        v.tensor_add(him[:], t3[:], t4[:])

                    # Pairwise repulsion S = sum_{j!=i} 1/(z_i - z_j),
                    # mirrored shifts: s and M-s terms share one reciprocal.
                    # Shifted reads come from doubled zx; mirror writes go to
                    # extended Sx (zeroed on ACT), folded into S afterwards.
                    sc.mul(Sxre[:], Sxre[:], 0.0)
                    sc.mul(Sxim[:], Sxim[:], 0.0)
                    for s in range(1, 17):
                        v.tensor_sub(V(t3), V(zre), VX(zxre)[:, :, s:s + M])
                        v.tensor_sub(V(t4), V(zim), VX(zxim)[:, :, s:s + M])
                        sc.square(sq1[:], t3[:])
                        sc.square(sq2[:], t4[:])
                        v.scalar_tensor_tensor(d2[:], sq1[:], SEPS, sq2[:],
                                               A.add, A.add)
                        v.reciprocal_approx_fast(rr[:], d2[:])
                        v.tensor_mul(t1[:], t3[:], rr[:])   # tre
                        v.tensor_mul(t2[:], t4[:], rr[:])   # tim
                        if s == 1:
                            sc.copy(Sre[:], t1[:])
                            sc.mul(Sim[:], t2[:], -1.0)
                        else:
                            v.tensor_add(Sre[:], Sre[:], t1[:])
                            v.tensor_sub(Sim[:], Sim[:], t2[:])
                        if s < 16:
                            v.tensor_sub(VX(Sxre)[:, :, s:s + M],
                                         VX(Sxre)[:, :, s:s + M], V(t1))
                            v.tensor_add(VX(Sxim)[:, :, s:s + M],
                                         VX(Sxim)[:, :, s:s + M], V(t2))
                    v.tensor_add(V(Sre), V(Sre), VX(Sxre)[:, :, 0:M])
                    v.tensor_add(V(Sre), V(Sre), VX(Sxre)[:, :, M:2 * M])
                    v.tensor_add(V(Sim), V(Sim), VX(Sxim)[:, :, 0:M])
                    v.tensor_add(V(Sim), V(Sim), VX(Sxim)[:, :, M:2 * M])

                    # joint scale-normalization of (p, p') by component max
                    # (|x| = max(-x, x); abs_max TT op not supported by codegen)
                    v.scalar_tensor_tensor(mx[:], hre[:], -1.0, hre[:],
                                           A.mult, A.max)
                    v.scalar_tensor_tensor(t1[:], him[:], -1.0, him[:],
                                           A.mult, A.max)
                    v.tensor_tensor(mx[:], mx[:], t1[:], op=A.max)
                    v.scalar_tensor_tensor(t1[:], qre[:], -1.0, qre[:],
                                           A.mult, A.max)
                    v.tensor_tensor(mx[:], mx[:], t1[:], op=A.max)
                    v.scalar_tensor_tensor(t1[:], qim[:], -1.0, qim[:],
                                           A.mult, A.max)
                    v.tensor_tensor(mx[:], mx[:], t1[:], op=A.max)
                    v.tensor_scalar_add(mx[:], mx[:], EPS)
                    v.reciprocal_approx_fast(mm[:], mx[:])
                    v.tensor_mul(hre[:], hre[:], mm[:])
                    v.tensor_mul(him[:], him[:], mm[:])
                    v.tensor_mul(qre[:], qre[:], mm[:])
                    v.tensor_mul(qim[:], qim[:], mm[:])
                    # den = p'n - pn*S
                    v.tensor_mul(t1[:], hre[:], Sre[:])
                    v.tensor_mul(t2[:], him[:], Sim[:])
                    v.tensor_mul(t3[:], hre[:], Sim[:])
                    v.tensor_mul(t4[:], him[:], Sre[:])
                    v.tensor_sub(dnre[:], qre[:], t1[:])
                    v.tensor_add(dnre[:], dnre[:], t2[:])
                    v.tensor_sub(dnim[:], qim[:], t3[:])
                    v.tensor_sub(dnim[:], dnim[:], t4[:])
                    sc.square(sq1[:], dnre[:])
                    sc.square(sq2[:], dnim[:])
                    v.scalar_tensor_tensor(d2[:], sq1[:], EPS, sq2[:],
                                           A.add, A.add)
                    v.reciprocal_approx_fast(rr[:], d2[:])
                    # w = pn * conj(den) / |den|^2
                    v.tensor_mul(t1[:], hre[:], dnre[:])
                    v.tensor_mul(t2[:], him[:], dnim[:])
                    v.tensor_add(t1[:], t1[:], t2[:])
                    v.tensor_mul(t3[:], him[:], dnre[:])
                    v.tensor_mul(t4[:], hre[:], dnim[:])
                    v.tensor_sub(t3[:], t3[:], t4[:])
                    v.tensor_mul(wre[:], t1[:], rr[:])
                    v.tensor_mul(wim[:], t3[:], rr[:])
                    # cap |w| <= 1
                    sc.square(sq1[:], wre[:])
                    sc.square(sq2[:], wim[:])
                    v.scalar_tensor_tensor(d2[:], sq1[:], EPS, sq2[:],
                                           A.add, A.add)
                    v.reciprocal_approx_fast(rr[:], d2[:])
                    sc.sqrt(rr2[:], rr[:])
                    v.tensor_scalar_min(mm[:], rr2[:], 1.0)
                    v.tensor_mul(wre[:], wre[:], mm[:])
                    v.tensor_mul(wim[:], wim[:], mm[:])
                    v.tensor_sub(zre[:], zre[:], wre[:])
                    v.tensor_sub(zim[:], zim[:], wim[:])
                    # clamp |z| <= R
                    sc.square(sq1[:], zre[:])
                    sc.square(sq2[:], zim[:])
                    v.scalar_tensor_tensor(d2[:], sq1[:], EPS, sq2[:],
                                           A.add, A.add)
                    v.reciprocal_approx_fast(rr[:], d2[:])
                    sc.sqrt(rr2[:], rr[:])
                    v.tensor_mul(mm[:], rr2[:], Rb[:])
                    v.tensor_scalar_min(mm[:], mm[:], 1.0)
                    v.tensor_mul(zre[:], zre[:], mm[:])
                    v.tensor_mul(zim[:], zim[:], mm[:])
                    if it < NITER - 1:
                        sc.copy(VX(zxre)[:, :, 0:M], V(zre))
                        sc.copy(VX(zxre)[:, :, M:2 * M], V(zre))
                        sc.copy(VX(zxim)[:, :, 0:M], V(zim))
                        sc.copy(VX(zxim)[:, :, M:2 * M], V(zim))

                nc.sync.dma_start(out=ore_d[ch * P:(ch + 1) * P], in_=zre[:])
                nc.sync.dma_start(out=oim_d[ch * P:(ch + 1) * P], in_=zim[:])

    nc.compile()
    _CACHE["nc"] = nc
    return nc


def _pack_plane(x):
    """(ROWS_CORE, M) -> (CH*P, F) with plane[p, g*M+i] = x[ch*G*P + g*P + p, i]."""
    return np.ascontiguousarray(
        x.reshape(CH, G, P, M).transpose(0, 2, 1, 3).reshape(CH * P, F))


def _unpack_plane(y):
    return y.reshape(CH, P, G, M).transpose(0, 2, 1, 3).reshape(ROWS_CORE, M)


def _pack_cb(c):
    """(ROWS_CORE, M) coeffs -> (CH*M*P, F), coeff k broadcast over the root axis."""
    cc = c.reshape(CH, G, P, M).transpose(0, 3, 2, 1)          # [ch, k, p, g]
    cc = np.broadcast_to(cc[..., None], (CH, M, P, G, M))
    return np.ascontiguousarray(cc.reshape(CH * M * P, F))


def _symmetrize_sort(z):
    """Pair conjugate roots exactly, zero near-real imags, canonical sort."""
    re = z.real.astype(np.float32)
    im = z.imag.astype(np.float32)
    tol = np.float32(1e-5)
    near_real = np.abs(im) <= tol * (1.0 + np.abs(re))
    K = np.where(near_real, 0.0, np.sign(im)).astype(np.float32)
    aim = np.where(near_real, np.float32(0.0), np.abs(im))
    idx = np.lexsort((aim, re, K), axis=-1)
    reS = np.take_along_axis(re, idx, -1)
    imS = np.take_along_axis(im, idx, -1)
    KS = np.take_along_axis(K, idx, -1)
    nN = (KS == -1).sum(-1, keepdims=True)
    nR = (KS == 0).sum(-1, keepdims=True)
    nP = (KS == 1).sum(-1, keepdims=True)
    ok = (nN == nP)
    j = np.arange(M)[None, :]
    isN = j < nN
    isP = j >= (nN + nR)
    partner = np.where(isN, j + nN + nR, np.where(isP, j - nN - nR, j))
    partner = np.clip(partner, 0, M - 1)
    reP = np.take_along_axis(reS, partner, -1)
    imP = np.take_along_axis(imS, partner, -1)
    mre = np.float32(0.5) * (reS + reP)
    sgn = np.where(isP, np.float32(1.0), np.where(isN, np.float32(-1.0),
                                                  np.float32(0.0)))
    mag = np.float32(0.5) * np.abs(imS - imP)
    mim = sgn * mag
    fre = np.where(ok, mre, reS).astype(np.float32)
    fim = np.where(ok, mim, imS).astype(np.float32)
    idx2 = np.lexsort((fim, fre), axis=-1)
    fre = np.take_along_axis(fre, idx2, -1)
    fim = np.take_along_axis(fim, idx2, -1)
    return (fre + 1j * fim).astype(np.complex64)


def kernel(a, _trace=False):
    import sys
    try:
        from concourse.bass_utils import run_bass_kernel_spmd
    except ImportError:
        sys.path.append("/opt/trn_rl_repo")
        from concourse.bass_utils import run_bass_kernel_spmd

    nc = _get_nc()
    a = np.asarray(a, dtype=np.float32)
    c = (-(a[:, 1:] / a[:, :1])).astype(np.float32)
    r = np.clip(np.abs(c[:, -1:]) ** (1.0 / M), 0.5, 2.0).astype(np.float32)
    ang = ((2.0 * np.pi / M) * np.arange(M, dtype=np.float32)
           + np.float32(0.4)).astype(np.float32)
    z0re = (r * np.cos(ang)[None, :]).astype(np.float32)
    z0im = (r * np.sin(ang)[None, :]).astype(np.float32)
    R = np.minimum(1.0 + np.max(np.abs(c), axis=1, keepdims=True),
                   11.0).astype(np.float32)
    Rb = np.broadcast_to(R, (B, M)).astype(np.float32)

    in_maps = []
    for core in range(NCORES):
        sl = slice(core * ROWS_CORE, (core + 1) * ROWS_CORE)
        in_maps.append({
            "cb": _pack_cb(c[sl]),
            "z0re": _pack_plane(z0re[sl]),
            "z0im": _pack_plane(z0im[sl]),
            "rb": _pack_plane(Rb[sl]),
        })
    out = run_bass_kernel_spmd(nc, in_maps, list(range(NCORES)), trace=_trace)
    if _trace:
        _CACHE["exec_time_ns"] = out.exec_time_ns
    res = out.results
    cores = []
    for core in range(NCORES):
        zre = _unpack_plane(res[core]["ore"])
        zim = _unpack_plane(res[core]["oim"])
        cores.append(zre + 1j * zim)
    z = np.concatenate(cores, axis=0).astype(np.complex64)
    return _symmetrize_sort(z)


# revision 20
# speedup vs baseline: 1.2870x; 1.1087x over previous
"""PolynomialToRoots on 8 Trainium2 NeuronCores.

Finds all 32 roots of 32768 degree-32 real polynomials (companion-matrix
eigenvalues) with a batched, guarded Ehrlich-Aberth iteration run entirely
on-device, data-parallel over the batch axis (4096 polynomials per core).

Output roots are conjugate-symmetrized and canonically sorted per row
(eigenvalue order of LAPACK geev is implementation-defined, so any
order-invariant comparison must sort; we return a deterministic canonical
order).
"""
import numpy as np

M = 32                     # polynomial degree / roots per row
P = 128                    # SBUF partitions
G = 16                     # row-groups per chunk
F = G * M                  # free dim: 512
CH = 2                     # chunks per core
ROWS_CORE = CH * G * P     # 4096
NCORES = 8
B = ROWS_CORE * NCORES     # 32768
NITER = 20
EPS = 1e-36
SEPS = 1e-30

_CACHE = {}


def _get_nc():
    if "nc" in _CACHE:
        return _CACHE["nc"]
    import sys
    try:
        import concourse.bacc as bacc
    except ImportError:
        sys.path.append("/opt/trn_rl_repo")
        import concourse.bacc as bacc
    import concourse.mybir as mybir
    from concourse.tile import TileContext

    A = mybir.AluOpType
    f32 = mybir.dt.float32
    nc = bacc.Bacc()
    cb_d = nc.dram_tensor("cb", [CH * M * P, F], f32, kind="ExternalInput")
    z0re_d = nc.dram_tensor("z0re", [CH * P, F], f32, kind="ExternalInput")
    z0im_d = nc.dram_tensor("z0im", [CH * P, F], f32, kind="ExternalInput")
    rb_d = nc.dram_tensor("rb", [CH * P, F], f32, kind="ExternalInput")
    ore_d = nc.dram_tensor("ore", [CH * P, F], f32, kind="ExternalOutput")
    oim_d = nc.dram_tensor("oim", [CH * P, F], f32, kind="ExternalOutput")

    with TileContext(nc) as tc:
        with tc.tile_pool(name="pool", bufs=1) as pool:
            def T(name):
                return pool.tile([P, F], f32, name=name)

            cb = [T("cb%d" % k) for k in range(M)]
            names = ("zre zim hre him qre qim Sre Sim Rb t1 t2 t3 t4 d2 rr "
                     "rr2 mx mm wre wim dnre dnim sq1 sq2").split()
            (zre, zim, hre, him, qre, qim, Sre, Sim, Rb, t1, t2, t3, t4, d2,
             rr, rr2, mx, mm, wre, wim, dnre, dnim, sq1, sq2) = (
                T(n) for n in names)
            # extended (doubled) planes: [p, g, 0:32] and [p, g, 32:64] both
            # hold z, so shifted reads z[i+s] and mirror writes S[i+s] need no
            # circular wrap-split.
            zxre = pool.tile([P, 2 * F], f32, name="zxre")
            zxim = pool.tile([P, 2 * F], f32, name="zxim")
            Sxre = pool.tile([P, 2 * F], f32, name="Sxre")
            Sxim = pool.tile([P, 2 * F], f32, name="Sxim")
            v = nc.vector
            sc = nc.scalar

            def V(t):
                return t[:].rearrange("p (g i) -> p g i", i=M)

            def VX(t):
                return t[:].rearrange("p (g i) -> p g i", i=2 * M)

            for ch in range(CH):
                for k in range(M):
                    r0 = (ch * M + k) * P
                    nc.sync.dma_start(out=cb[k][:], in_=cb_d[r0:r0 + P])
                nc.sync.dma_start(out=zre[:], in_=z0re_d[ch * P:(ch + 1) * P])
                nc.sync.dma_start(out=zim[:], in_=z0im_d[ch * P:(ch + 1) * P])
                nc.sync.dma_start(out=Rb[:], in_=rb_d[ch * P:(ch + 1) * P])
                # zx = [z, z] per group (ACT engine; DVE stays busy)
                sc.copy(VX(zxre)[:, :, 0:M], V(zre))
                sc.copy(VX(zxre)[:, :, M:2 * M], V(zre))
                sc.copy(VX(zxim)[:, :, 0:M], V(zim))
                sc.copy(VX(zxim)[:, :, M:2 * M], V(zim))
                # exact zero init (uninitialized SBUF may hold NaN; x*0 would
                # keep it NaN, z-z cannot)
                for half in (slice(0, M), slice(M, 2 * M)):
                    v.tensor_sub(VX(Sxre)[:, :, half], V(zre), V(zre))
                    v.tensor_sub(VX(Sxim)[:, :, half], V(zre), V(zre))

                for it in range(NITER):
                    # Horner for p (h) and p' (q).  h steps singly (c_k real,
                    # 7 ops); q steps in fused pairs via
                    #   q_{j+2} = q_j z^2 + 2 h_{j+1} + c_{j+1}
                    # (since h_j z = h_{j+1} + c_{j+1}), 9 ops per pair.
                    # z^2 in (wre, wim) — free until the tail.

                    def hstep(k):
                        v.tensor_mul(t1[:], hre[:], zre[:])
                        v.tensor_mul(t2[:], him[:], zim[:])
                        v.tensor_mul(t3[:], hre[:], zim[:])
                        v.tensor_mul(t4[:], him[:], zre[:])
                        v.tensor_sub(hre[:], t1[:], t2[:])
                        v.tensor_sub(hre[:], hre[:], cb[k][:])
                        v.tensor_add(him[:], t3[:], t4[:])

                    v.tensor_mul(t1[:], zre[:], zre[:])
                    v.tensor_mul(t2[:], zim[:], zim[:])
                    v.tensor_sub(wre[:], t1[:], t2[:])
                    v.tensor_mul(t3[:], zre[:], zim[:])
                    v.tensor_add(wim[:], t3[:], t3[:])
                    # h0 = z - c0; q1 = z + h0
                    v.tensor_sub(hre[:], zre[:], cb[0][:])
                    sc.copy(him[:], zim[:])
                    v.tensor_add(qre[:], zre[:], hre[:])
                    v.tensor_add(qim[:], zim[:], him[:])
                    hstep(1)
                    for j in range(1, M - 2, 2):
                        hstep(j + 1)
                        v.tensor_mul(t1[:], qre[:], wre[:])
                        v.tensor_mul(t2[:], qim[:], wim[:])
                        v.tensor_mul(t3[:], qre[:], wim[:])
                        v.tensor_mul(t4[:], qim[:], wre[:])
                        v.tensor_sub(t1[:], t1[:], t2[:])
                        v.tensor_add(t3[:], t3[:], t4[:])
                        v.scalar_tensor_tensor(qre[:], hre[:], 2.0, t1[:],
                                               A.mult, A.add)
                        v.tensor_add(qre[:], qre[:], cb[j + 1][:])
                        v.scalar_tensor_tensor(qim[:], him[:], 2.0, t3[:],
                                               A.mult, A.add)
                        hstep(j + 2)

                    # Pairwise repulsion S = sum_{j!=i} 1/(z_i - z_j),
                    # mirrored shifts: s and M-s terms share one reciprocal.
                    # Shifted reads come from doubled zx; mirror writes go to
                    # extended Sx (zeroed on ACT), folded into S afterwards.
                    sc.mul(Sxre[:], Sxre[:], 0.0)
                    sc.mul(Sxim[:], Sxim[:], 0.0)
                    for s in range(1, 17):
                        v.tensor_sub(V(t3), V(zre), VX(zxre)[:, :, s:s + M])
                        v.tensor_sub(V(t4), V(zim), VX(zxim)[:, :, s:s + M])
                        sc.square(sq1[:], t3[:])
                        sc.square(sq2[:], t4[:])
                        v.scalar_tensor_tensor(d2[:], sq1[:], SEPS, sq2[:],
                                               A.add, A.add)
                        v.reciprocal_approx_fast(rr[:], d2[:])
                        v.tensor_mul(t1[:], t3[:], rr[:])   # tre
                        v.tensor_mul(t2[:], t4[:], rr[:])   # tim
                        if s == 1:
                            sc.copy(Sre[:], t1[:])
                            sc.mul(Sim[:], t2[:], -1.0)
                        else:
                            v.tensor_add(Sre[:], Sre[:], t1[:])
                            v.tensor_sub(Sim[:], Sim[:], t2[:])
                        if s < 16:
                            v.tensor_sub(VX(Sxre)[:, :, s:s + M],
                                         VX(Sxre)[:, :, s:s + M], V(t1))
                            v.tensor_add(VX(Sxim)[:, :, s:s + M],
                                         VX(Sxim)[:, :, s:s + M], V(t2))
                    v.tensor_add(V(Sre), V(Sre), VX(Sxre)[:, :, 0:M])
                    v.tensor_add(V(Sre), V(Sre), VX(Sxre)[:, :, M:2 * M])
                    v.tensor_add(V(Sim), V(Sim), VX(Sxim)[:, :, 0:M])
                    v.tensor_add(V(Sim), V(Sim), VX(Sxim)[:, :, M:2 * M])

                    # joint scale-normalization of (p, p') by component max
                    # (|x| = max(-x, x); abs_max TT op not supported by codegen)
                    v.scalar_tensor_tensor(mx[:], hre[:], -1.0, hre[:],
                                           A.mult, A.max)
                    v.scalar_tensor_tensor(t1[:], him[:], -1.0, him[:],
                                           A.mult, A.max)
                    v.tensor_tensor(mx[:], mx[:], t1[:], op=A.max)
                    v.scalar_tensor_tensor(t1[:], qre[:], -1.0, qre[:],
                                           A.mult, A.max)
                    v.tensor_tensor(mx[:], mx[:], t1[:], op=A.max)
                    v.scalar_tensor_tensor(t1[:], qim[:], -1.0, qim[:],
                                           A.mult, A.max)
                    v.tensor_tensor(mx[:], mx[:], t1[:], op=A.max)
                    v.tensor_scalar_add(mx[:], mx[:], EPS)
                    v.reciprocal_approx_fast(mm[:], mx[:])
                    v.tensor_mul(hre[:], hre[:], mm[:])
                    v.tensor_mul(him[:], him[:], mm[:])
                    v.tensor_mul(qre[:], qre[:], mm[:])
                    v.tensor_mul(qim[:], qim[:], mm[:])
                    # den = p'n - pn*S
                    v.tensor_mul(t1[:], hre[:], Sre[:])
                    v.tensor_mul(t2[:], him[:], Sim[:])
                    v.tensor_mul(t3[:], hre[:], Sim[:])
                    v.tensor_mul(t4[:], him[:], Sre[:])
                    v.tensor_sub(dnre[:], qre[:], t1[:])
                    v.tensor_add(dnre[:], dnre[:], t2[:])
                    v.tensor_sub(dnim[:], qim[:], t3[:])
                    v.tensor_sub(dnim[:], dnim[:], t4[:])
                    sc.square(sq1[:], dnre[:])
                    sc.square(sq2[:], dnim[:])
                    v.scalar_tensor_tensor(d2[:], sq1[:], EPS, sq2[:],
                                           A.add, A.add)
                    v.reciprocal_approx_fast(rr[:], d2[:])
                    # w = pn * conj(den) / |den|^2
                    v.tensor_mul(t1[:], hre[:], dnre[:])
                    v.tensor_mul(t2[:], him[:], dnim[:])
                    v.tensor_add(t1[:], t1[:], t2[:])
                    v.tensor_mul(t3[:], him[:], dnre[:])
                    v.tensor_mul(t4[:], hre[:], dnim[:])
                    v.tensor_sub(t3[:], t3[:], t4[:])
                    v.tensor_mul(wre[:], t1[:], rr[:])
                    v.tensor_mul(wim[:], t3[:], rr[:])
                    # cap |w| <= 1
                    sc.square(sq1[:], wre[:])
                    sc.square(sq2[:], wim[:])
                    v.scalar_tensor_tensor(d2[:], sq1[:], EPS, sq2[:],
                                           A.add, A.add)
                    v.reciprocal_approx_fast(rr[:], d2[:])
                    sc.sqrt(rr2[:], rr[:])
                    v.tensor_scalar_min(mm[:], rr2[:], 1.0)
                    v.tensor_mul(wre[:], wre[:], mm[:])
                    v.tensor_mul(wim[:], wim[:], mm[:])
                    v.tensor_sub(zre[:], zre[:], wre[:])
                    v.tensor_sub(zim[:], zim[:], wim[:])
                    # clamp |z| <= R
                    sc.square(sq1[:], zre[:])
                    sc.square(sq2[:], zim[:])
                    v.scalar_tensor_tensor(d2[:], sq1[:], EPS, sq2[:],
                                           A.add, A.add)
                    v.reciprocal_approx_fast(rr[:], d2[:])
                    sc.sqrt(rr2[:], rr[:])
                    v.tensor_mul(mm[:], rr2[:], Rb[:])
                    v.tensor_scalar_min(mm[:], mm[:], 1.0)
                    v.tensor_mul(zre[:], zre[:], mm[:])
                    v.tensor_mul(zim[:], zim[:], mm[:])
                    if it < NITER - 1:
                        sc.copy(VX(zxre)[:, :, 0:M], V(zre))
                        sc.copy(VX(zxre)[:, :, M:2 * M], V(zre))
                        sc.copy(VX(zxim)[:, :, 0:M], V(zim))
                        sc.copy(VX(zxim)[:, :, M:2 * M], V(zim))

                nc.sync.dma_start(out=ore_d[ch * P:(ch + 1) * P], in_=zre[:])
                nc.sync.dma_start(out=oim_d[ch * P:(ch + 1) * P], in_=zim[:])

    nc.compile()
    _CACHE["nc"] = nc
    return nc


def _pack_plane(x):
    """(ROWS_CORE, M) -> (CH*P, F) with plane[p, g*M+i] = x[ch*G*P + g*P + p, i]."""
    return np.ascontiguousarray(
        x.reshape(CH, G, P, M).transpose(0, 2, 1, 3).reshape(CH * P, F))


def _unpack_plane(y):
    return y.reshape(CH, P, G, M).transpose(0, 2, 1, 3).reshape(ROWS_CORE, M)


def _pack_cb(c):
    """(ROWS_CORE, M) coeffs -> (CH*M*P, F), coeff k broadcast over the root axis."""
    cc = c.reshape(CH, G, P, M).transpose(0, 3, 2, 1)          # [ch, k, p, g]
    cc = np.broadcast_to(cc[..., None], (CH, M, P, G, M))
    return np.ascontiguousarray(cc.reshape(CH * M * P, F))


def _symmetrize_sort(z):
    """Pair conjugate roots exactly, zero near-real imags, canonical sort."""
    re = z.real.astype(np.float32)
    im = z.imag.astype(np.float32)
    tol = np.float32(1e-5)
    near_real = np.abs(im) <= tol * (1.0 + np.abs(re))
    K = np.where(near_real, 0.0, np.sign(im)).astype(np.float32)
    aim = np.where(near_real, np.float32(0.0), np.abs(im))
    idx = np.lexsort((aim, re, K), axis=-1)
    reS = np.take_along_axis(re, idx, -1)
    imS = np.take_along_axis(im, idx, -1)
    KS = np.take_along_axis(K, idx, -1)
    nN = (KS == -1).sum(-1, keepdims=True)
    nR = (KS == 0).sum(-1, keepdims=True)
    nP = (KS == 1).sum(-1, keepdims=True)
    ok = (nN == nP)
    j = np.arange(M)[None, :]
    isN = j < nN
    isP = j >= (nN + nR)
    partner = np.where(isN, j + nN + nR, np.where(isP, j - nN - nR, j))
    partner = np.clip(partner, 0, M - 1)
    reP = np.take_along_axis(reS, partner, -1)
    imP = np.take_along_axis(imS, partner, -1)
    mre = np.float32(0.5) * (reS + reP)
    sgn = np.where(isP, np.float32(1.0), np.where(isN, np.float32(-1.0),
                                                  np.float32(0.0)))
    mag = np.float32(0.5) * np.abs(imS - imP)
    mim = sgn * mag
    fre = np.where(ok, mre, reS).astype(np.float32)
    fim = np.where(ok, mim, imS).astype(np.float32)
    idx2 = np.lexsort((fim, fre), axis=-1)
    fre = np.take_along_axis(fre, idx2, -1)
    fim = np.take_along_axis(fim, idx2, -1)
    return (fre + 1j * fim).astype(np.complex64)


def kernel(a, _trace=False):
    import sys
    try:
        from concourse.bass_utils import run_bass_kernel_spmd
    except ImportError:
        sys.path.append("/opt/trn_rl_repo")
        from concourse.bass_utils import run_bass_kernel_spmd

    nc = _get_nc()
    a = np.asarray(a, dtype=np.float32)
    c = (-(a[:, 1:] / a[:, :1])).astype(np.float32)
    r = np.clip(np.abs(c[:, -1:]) ** (1.0 / M), 0.5, 2.0).astype(np.float32)
    ang = ((2.0 * np.pi / M) * np.arange(M, dtype=np.float32)
           + np.float32(0.4)).astype(np.float32)
    z0re = (r * np.cos(ang)[None, :]).astype(np.float32)
    z0im = (r * np.sin(ang)[None, :]).astype(np.float32)
    R = np.minimum(1.0 + np.max(np.abs(c), axis=1, keepdims=True),
                   11.0).astype(np.float32)
    Rb = np.broadcast_to(R, (B, M)).astype(np.float32)

    in_maps = []
    for core in range(NCORES):
        sl = slice(core * ROWS_CORE, (core + 1) * ROWS_CORE)
        in_maps.append({
            "cb": _pack_cb(c[sl]),
            "z0re": _pack_plane(z0re[sl]),
            "z0im": _pack_plane(z0im[sl]),
            "rb": _pack_plane(Rb[sl]),
        })
    out = run_bass_kernel_spmd(nc, in_maps, list(range(NCORES)), trace=_trace)
    if _trace:
        _CACHE["exec_time_ns"] = out.exec_time_ns
    res = out.results
    cores = []
    for core in range(NCORES):
        zre = _unpack_plane(res[core]["ore"])
        zim = _unpack_plane(res[core]["oim"])
        cores.append(zre + 1j * zim)
    z = np.concatenate(cores, axis=0).astype(np.complex64)
    return _symmetrize_sort(z)


# revision 34
# speedup vs baseline: 1.4553x; 1.1308x over previous
"""PolynomialToRoots on 8 Trainium2 NeuronCores.

Finds all 32 roots of 32768 degree-32 real polynomials (companion-matrix
eigenvalues) with a batched, guarded Ehrlich-Aberth iteration run entirely
on-device, data-parallel over the batch axis (4096 polynomials per core).

Output roots are conjugate-symmetrized and canonically sorted per row
(eigenvalue order of LAPACK geev is implementation-defined, so any
order-invariant comparison must sort; we return a deterministic canonical
order).
"""
import numpy as np

M = 32                     # polynomial degree / roots per row
P = 128                    # SBUF partitions
G = 32                     # row-groups per chunk
F = G * M                  # free dim: 1024
CH = 1                     # chunks per core
ROWS_CORE = CH * G * P     # 4096
NCORES = 8
B = ROWS_CORE * NCORES     # 32768
NITER = 20
EPS = 1e-36
SEPS = 1e-30

_CACHE = {}


def _get_nc():
    if "nc" in _CACHE:
        return _CACHE["nc"]
    import sys
    try:
        import concourse.bacc as bacc
    except ImportError:
        sys.path.append("/opt/trn_rl_repo")
        import concourse.bacc as bacc
    import concourse.mybir as mybir
    from concourse.tile import TileContext

    A = mybir.AluOpType
    AF = mybir.ActivationFunctionType
    f32 = mybir.dt.float32
    nc = bacc.Bacc()
    cb_d = nc.dram_tensor("cb", [CH * M * P, G], f32, kind="ExternalInput")
    z0re_d = nc.dram_tensor("z0re", [CH * P, F], f32, kind="ExternalInput")
    z0im_d = nc.dram_tensor("z0im", [CH * P, F], f32, kind="ExternalInput")
    rb_d = nc.dram_tensor("rb", [CH * P, F], f32, kind="ExternalInput")
    ore_d = nc.dram_tensor("ore", [CH * P, F], f32, kind="ExternalOutput")
    oim_d = nc.dram_tensor("oim", [CH * P, F], f32, kind="ExternalOutput")

    with TileContext(nc) as tc:
        with tc.tile_pool(name="pool", bufs=1) as pool:
            def T(name):
                return pool.tile([P, F], f32, name=name)

            # coefficients stored compactly [P, G] (constant across the 32
            # roots of a group) and read through 0-stride broadcast APs
            cb = [pool.tile([P, G], f32, name="cb%d" % k) for k in range(M)]

            def cbb(k):
                return cb[k][:].broadcast_to((P, G, M))
            names = ("zre zim hre him qre qim Sre Sim Rb t1 t2 t3 t4 d2 rr "
                     "rr2 mx mm wre wim dnre dnim sq1 sq2 dre2 dim2").split()
            (zre, zim, hre, him, qre, qim, Sre, Sim, Rb, t1, t2, t3, t4, d2,
             rr, rr2, mx, mm, wre, wim, dnre, dnim, sq1, sq2, dre2, dim2) = (
                T(n) for n in names)
            # extended (doubled) planes: [p, g, 0:32] and [p, g, 32:64] both
            # hold z, so shifted reads z[i+s] and mirror writes S[i+s] need no
            # circular wrap-split.
            zxre = pool.tile([P, 2 * F], f32, name="zxre")
            zxim = pool.tile([P, 2 * F], f32, name="zxim")
            Sxre = pool.tile([P, 2 * F], f32, name="Sxre")
            Sxim = pool.tile([P, 2 * F], f32, name="Sxim")
            v = nc.vector
            sc = nc.scalar

            def V(t):
                return t[:].rearrange("p (g i) -> p g i", i=M)

            def VX(t):
                return t[:].rearrange("p (g i) -> p g i", i=2 * M)

            for ch in range(CH):
                for k in range(M):
                    r0 = (ch * M + k) * P
                    nc.sync.dma_start(out=cb[k][:], in_=cb_d[r0:r0 + P])
                nc.sync.dma_start(out=zre[:], in_=z0re_d[ch * P:(ch + 1) * P])
                nc.sync.dma_start(out=zim[:], in_=z0im_d[ch * P:(ch + 1) * P])
                nc.sync.dma_start(out=Rb[:], in_=rb_d[ch * P:(ch + 1) * P])
                # zx = [z, z] per group (ACT engine; DVE stays busy)
                sc.copy(VX(zxre)[:, :, 0:M], V(zre))
                sc.copy(VX(zxre)[:, :, M:2 * M], V(zre))
                sc.copy(VX(zxim)[:, :, 0:M], V(zim))
                sc.copy(VX(zxim)[:, :, M:2 * M], V(zim))
                # exact zero init (uninitialized SBUF may hold NaN; x*0 would
                # keep it NaN, z-z cannot)
                for half in (slice(0, M), slice(M, 2 * M)):
                    v.tensor_sub(VX(Sxre)[:, :, half], V(zre), V(zre))
                    v.tensor_sub(VX(Sxim)[:, :, half], V(zre), V(zre))

                for it in range(NITER):
                    # Horner for p (h) and p' (q).  h steps singly (c_k real,
                    # 7 ops); q steps in fused pairs via
                    #   q_{j+2} = q_j z^2 + 2 h_{j+1} + c_{j+1}
                    # (since h_j z = h_{j+1} + c_{j+1}), 9 ops per pair.
                    # z^2 in (wre, wim) — free until the tail.

                    def hstep(k):
                        v.tensor_mul(t1[:], hre[:], zre[:])
                        v.tensor_mul(t2[:], him[:], zim[:])
                        v.tensor_mul(t3[:], hre[:], zim[:])
                        v.tensor_mul(t4[:], him[:], zre[:])
                        v.tensor_sub(hre[:], t1[:], t2[:])
                        v.tensor_sub(V(hre), V(hre), cbb(k))
                        v.tensor_add(him[:], t3[:], t4[:])

                    # Sx zeroing + shift-1 diff/squares hoisted up here so the
                    # ACT work lands while DVE grinds through Horner (S-sum
                    # depends only on z/zx, which are fixed for the iteration).
                    sc.mul(Sxre[:], Sxre[:], 0.0)
                    sc.mul(Sxim[:], Sxim[:], 0.0)
                    v.tensor_mul(t1[:], zre[:], zre[:])
                    v.tensor_mul(t2[:], zim[:], zim[:])
                    v.tensor_sub(wre[:], t1[:], t2[:])
                    v.tensor_mul(t3[:], zre[:], zim[:])
                    v.tensor_add(wim[:], t3[:], t3[:])
                    # h0 = (z - c0, zim); q1 = z + h0.  h0.im == zim is used
                    # directly (no copy) to keep ACT off the critical path.
                    v.tensor_sub(V(hre), V(zre), cbb(0))
                    v.tensor_add(qre[:], zre[:], hre[:])
                    v.tensor_add(qim[:], zim[:], zim[:])
                    # hstep(1) with him==zim inlined
                    v.tensor_mul(t1[:], hre[:], zre[:])
                    v.tensor_mul(t2[:], zim[:], zim[:])
                    v.tensor_mul(t3[:], hre[:], zim[:])
                    v.tensor_mul(t4[:], zim[:], zre[:])
                    v.tensor_sub(hre[:], t1[:], t2[:])
                    v.tensor_sub(V(hre), V(hre), cbb(1))
                    v.tensor_add(him[:], t3[:], t4[:])
                    v.tensor_sub(V(dre2), V(zre), VX(zxre)[:, :, 1:1 + M])
                    v.tensor_sub(V(dim2), V(zim), VX(zxim)[:, :, 1:1 + M])
                    sc.square(sq1[:], dre2[:])
                    sc.square(sq2[:], dim2[:])
                    for j in range(1, M - 2, 2):
                        hstep(j + 1)
                        v.tensor_mul(t1[:], qre[:], wre[:])
                        v.tensor_mul(t2[:], qim[:], wim[:])
                        v.tensor_mul(t3[:], qre[:], wim[:])
                        v.tensor_mul(t4[:], qim[:], wre[:])
                        v.tensor_sub(t1[:], t1[:], t2[:])
                        v.tensor_add(t3[:], t3[:], t4[:])
                        v.scalar_tensor_tensor(qre[:], hre[:], 2.0, t1[:],
                                               A.mult, A.add)
                        v.tensor_add(V(qre), V(qre), cbb(j + 1))
                        v.scalar_tensor_tensor(qim[:], him[:], 2.0, t3[:],
                                               A.mult, A.add)
                        hstep(j + 2)

                    # |h|,|q| components on ACT (exact), overlapping the S-sum;
                    # mm/mx/dn tiles are free until the tail.
                    sc.activation(mx[:], hre[:], AF.Abs)
                    sc.activation(mm[:], him[:], AF.Abs)
                    sc.activation(dnre[:], qre[:], AF.Abs)
                    sc.activation(dnim[:], qim[:], AF.Abs)

                    # Pairwise repulsion S = sum_{j!=i} 1/(z_i - z_j),
                    # mirrored shifts: s and M-s terms share one reciprocal.
                    # Shifted reads come from doubled zx; mirror writes go to
                    # extended Sx (zeroed on ACT), folded into S afterwards.
                    # Software-pipelined: shift s+1's diffs are issued mid-way
                    # through shift s so ACT squares them while DVE finishes s
                    # (odd shifts live in dre2/dim2, even in t3/t4).
                    for s in range(1, 17):
                        da, db = (dre2, dim2) if s % 2 == 1 else (t3, t4)
                        na, nb = (t3, t4) if s % 2 == 1 else (dre2, dim2)
                        v.scalar_tensor_tensor(d2[:], sq1[:], SEPS, sq2[:],
                                               A.add, A.add)
                        if s < 16:
                            v.tensor_sub(V(na), V(zre),
                                         VX(zxre)[:, :, s + 1:s + 1 + M])
                            v.tensor_sub(V(nb), V(zim),
                                         VX(zxim)[:, :, s + 1:s + 1 + M])
                            sc.square(sq1[:], na[:])
                            sc.square(sq2[:], nb[:])
                        v.reciprocal_approx_fast(rr[:], d2[:])
                        v.tensor_mul(t1[:], da[:], rr[:])   # tre
                        v.tensor_mul(t2[:], db[:], rr[:])   # tim
                        if s == 1:
                            sc.copy(Sre[:], t1[:])
                            sc.mul(Sim[:], t2[:], -1.0)
                        else:
                            v.tensor_add(Sre[:], Sre[:], t1[:])
                            v.tensor_sub(Sim[:], Sim[:], t2[:])
                        if s < 16:
                            v.tensor_sub(VX(Sxre)[:, :, s:s + M],
                                         VX(Sxre)[:, :, s:s + M], V(t1))
                            v.tensor_add(VX(Sxim)[:, :, s:s + M],
                                         VX(Sxim)[:, :, s:s + M], V(t2))
                    v.tensor_add(V(Sre), V(Sre), VX(Sxre)[:, :, 0:M])
                    v.tensor_add(V(Sre), V(Sre), VX(Sxre)[:, :, M:2 * M])
                    v.tensor_add(V(Sim), V(Sim), VX(Sxim)[:, :, 0:M])
                    v.tensor_add(V(Sim), V(Sim), VX(Sxim)[:, :, M:2 * M])

                    # joint scale-normalization of (p, p') by component max
                    # (abs values already computed on ACT during the S-sum)
                    v.tensor_tensor(mx[:], mx[:], mm[:], op=A.max)
                    v.tensor_tensor(dnre[:], dnre[:], dnim[:], op=A.max)
                    v.tensor_tensor(mx[:], mx[:], dnre[:], op=A.max)
                    v.tensor_scalar_add(mx[:], mx[:], EPS)
                    v.reciprocal_approx_fast(mm[:], mx[:])
                    v.tensor_mul(hre[:], hre[:], mm[:])
                    v.tensor_mul(him[:], him[:], mm[:])
                    v.tensor_mul(qre[:], qre[:], mm[:])
                    v.tensor_mul(qim[:], qim[:], mm[:])
                    # den = p'n - pn*S
                    v.tensor_mul(t1[:], hre[:], Sre[:])
                    v.tensor_mul(t2[:], him[:], Sim[:])
                    v.tensor_mul(t3[:], hre[:], Sim[:])
                    v.tensor_mul(t4[:], him[:], Sre[:])
                    v.tensor_sub(dnre[:], qre[:], t1[:])
                    v.tensor_add(dnre[:], dnre[:], t2[:])
                    v.tensor_sub(dnim[:], qim[:], t3[:])
                    v.tensor_sub(dnim[:], dnim[:], t4[:])
                    sc.square(sq1[:], dnre[:])
                    sc.square(sq2[:], dnim[:])
                    v.scalar_tensor_tensor(d2[:], sq1[:], EPS, sq2[:],
                                           A.add, A.add)
                    v.reciprocal_approx_fast(rr[:], d2[:])
                    # w = pn * conj(den) / |den|^2
                    v.tensor_mul(t1[:], hre[:], dnre[:])
                    v.tensor_mul(t2[:], him[:], dnim[:])
                    v.tensor_add(t1[:], t1[:], t2[:])
                    v.tensor_mul(t3[:], him[:], dnre[:])
                    v.tensor_mul(t4[:], hre[:], dnim[:])
                    v.tensor_sub(t3[:], t3[:], t4[:])
                    v.tensor_mul(wre[:], t1[:], rr[:])
                    v.tensor_mul(wim[:], t3[:], rr[:])
                    # cap |w| <= 1
                    sc.square(sq1[:], wre[:])
                    sc.square(sq2[:], wim[:])
                    v.scalar_tensor_tensor(d2[:], sq1[:], EPS, sq2[:],
                                           A.add, A.add)
                    v.reciprocal_approx_fast(rr[:], d2[:])
                    sc.sqrt(rr2[:], rr[:])
                    v.tensor_scalar_min(mm[:], rr2[:], 1.0)
                    v.tensor_mul(wre[:], wre[:], mm[:])
                    v.tensor_mul(wim[:], wim[:], mm[:])
                    v.tensor_sub(zre[:], zre[:], wre[:])
                    v.tensor_sub(zim[:], zim[:], wim[:])
                    # clamp |z| <= R
                    sc.square(sq1[:], zre[:])
                    sc.square(sq2[:], zim[:])
                    v.scalar_tensor_tensor(d2[:], sq1[:], EPS, sq2[:],
                                           A.add, A.add)
                    v.reciprocal_approx_fast(rr[:], d2[:])
                    sc.sqrt(rr2[:], rr[:])
                    v.tensor_mul(mm[:], rr2[:], Rb[:])
                    v.tensor_scalar_min(mm[:], mm[:], 1.0)
                    v.tensor_mul(zre[:], zre[:], mm[:])
                    v.tensor_mul(zim[:], zim[:], mm[:])
                    if it < NITER - 1:
                        sc.copy(VX(zxre)[:, :, 0:M], V(zre))
                        sc.copy(VX(zxre)[:, :, M:2 * M], V(zre))
                        sc.copy(VX(zxim)[:, :, 0:M], V(zim))
                        sc.copy(VX(zxim)[:, :, M:2 * M], V(zim))

                nc.sync.dma_start(out=ore_d[ch * P:(ch + 1) * P], in_=zre[:])
                nc.sync.dma_start(out=oim_d[ch * P:(ch + 1) * P], in_=zim[:])

    nc.compile()
    _CACHE["nc"] = nc
    return nc


def _pack_plane(x):
    """(ROWS_CORE, M) -> (CH*P, F) with plane[p, g*M+i] = x[ch*G*P + g*P + p, i]."""
    return np.ascontiguousarray(
        x.reshape(CH, G, P, M).transpose(0, 2, 1, 3).reshape(CH * P, F))


def _unpack_plane(y):
    return y.reshape(CH, P, G, M).transpose(0, 2, 1, 3).reshape(ROWS_CORE, M)


def _pack_cb(c):
    """(ROWS_CORE, M) coeffs -> (CH*M*P, G); root-axis broadcast happens on
    device via 0-stride APs."""
    cc = c.reshape(CH, G, P, M).transpose(0, 3, 2, 1)          # [ch, k, p, g]
    return np.ascontiguousarray(cc.reshape(CH * M * P, G))


def _symmetrize_sort(z):
    """Pair conjugate roots exactly, zero near-real imags, canonical sort."""
    re = z.real.astype(np.float32)
    im = z.imag.astype(np.float32)
    tol = np.float32(1e-5)
    near_real = np.abs(im) <= tol * (1.0 + np.abs(re))
    K = np.where(near_real, 0.0, np.sign(im)).astype(np.float32)
    aim = np.where(near_real, np.float32(0.0), np.abs(im))
    idx = np.lexsort((aim, re, K), axis=-1)
    reS = np.take_along_axis(re, idx, -1)
    imS = np.take_along_axis(im, idx, -1)
    KS = np.take_along_axis(K, idx, -1)
    nN = (KS == -1).sum(-1, keepdims=True)
    nR = (KS == 0).sum(-1, keepdims=True)
    nP = (KS == 1).sum(-1, keepdims=True)
    ok = (nN == nP)
    j = np.arange(M)[None, :]
    isN = j < nN
    isP = j >= (nN + nR)
    partner = np.where(isN, j + nN + nR, np.where(isP, j - nN - nR, j))
    partner = np.clip(partner, 0, M - 1)
    reP = np.take_along_axis(reS, partner, -1)
    imP = np.take_along_axis(imS, partner, -1)
    mre = np.float32(0.5) * (reS + reP)
    sgn = np.where(isP, np.float32(1.0), np.where(isN, np.float32(-1.0),
                                                  np.float32(0.0)))
    mag = np.float32(0.5) * np.abs(imS - imP)
    mim = sgn * mag
    fre = np.where(ok, mre, reS).astype(np.float32)
    fim = np.where(ok, mim, imS).astype(np.float32)
    idx2 = np.lexsort((fim, fre), axis=-1)
    fre = np.take_along_axis(fre, idx2, -1)
    fim = np.take_along_axis(fim, idx2, -1)
    return (fre + 1j * fim).astype(np.complex64)


def kernel(a, _trace=False):
    import sys
    try:
        from concourse.bass_utils import run_bass_kernel_spmd
    except ImportError:
        sys.path.append("/opt/trn_rl_repo")
        from concourse.bass_utils import run_bass_kernel_spmd

    nc = _get_nc()
    a = np.asarray(a, dtype=np.float32)
    c = (-(a[:, 1:] / a[:, :1])).astype(np.float32)
    r = np.clip(np.abs(c[:, -1:]) ** (1.0 / M), 0.5, 2.0).astype(np.float32)
    ang = ((2.0 * np.pi / M) * np.arange(M, dtype=np.float32)
           + np.float32(0.4)).astype(np.float32)
    z0re = (r * np.cos(ang)[None, :]).astype(np.float32)
    z0im = (r * np.sin(ang)[None, :]).astype(np.float32)
    R = np.minimum(1.0 + np.max(np.abs(c), axis=1, keepdims=True),
                   11.0).astype(np.float32)
    Rb = np.broadcast_to(R, (B, M)).astype(np.float32)

    in_maps = []
    for core in range(NCORES):
        sl = slice(core * ROWS_CORE, (core + 1) * ROWS_CORE)
        in_maps.append({
            "cb": _pack_cb(c[sl]),
            "z0re": _pack_plane(z0re[sl]),
            "z0im": _pack_plane(z0im[sl]),
            "rb": _pack_plane(Rb[sl]),
        })
    out = run_bass_kernel_spmd(nc, in_maps, list(range(NCORES)), trace=_trace)
    if _trace:
        _CACHE["exec_time_ns"] = out.exec_time_ns
    res = out.results
    cores = []
    for core in range(NCORES):
        zre = _unpack_plane(res[core]["ore"])
        zim = _unpack_plane(res[core]["oim"])
        cores.append(zre + 1j * zim)
    z = np.concatenate(cores, axis=0).astype(np.complex64)
    return _symmetrize_sort(z)
